# revision 38
# baseline (speedup 1.0000x reference)
"""Trainium2 Bass kernel for nn_CrossDomainAspectLabelPropagation.

Data-parallel MoE-routed implementation on 8 NeuronCores:
  - Host groups rows by domain and packs them into per-core "weight slots"
    (each (core, slot) processes rows of a single domain), so each core runs
    its expert MLP only once per row instead of densely over all 5 experts.
  - On-device layout is feature-major (activations stored transposed,
    [hidden, rows]): weights are the stationary matmul operand, activations
    the moving one, so no transposes are ever needed on device; the host
    pre/post-transposes.
  - All matmuls run in float32r (TF32-like, ~1e-4 rel err) at full PE rate.
  - LayerNorm needs no mean on device (the host pre-centers the columns of
    the preceding Linear, which is exact math); variance comes from
    squares (DVE) + ones-vector matmuls, rstd via ACT sqrt +
    reciprocal_approx_fast, broadcast with gpsimd partition_broadcast.
    For the invariant branch, gamma is host-folded into iW2 and beta is
    deferred to the combined-output copy (consumers carry W.T@beta in their
    biases), which halves the LN critical chain before disc/cls.
  - PSUM drains run on the ACT engine (fused bias+relu), squares/applies on
    DVE, broadcasts on GpSimd - balancing all four compute engines.
  - The domain loss is computed per-row on device; the host sums the 8
    partial vectors.

Measured on 8 axon-attached TRN2 NeuronCores: ~214 us HW exec time,
worst-output relative error ~3.9e-4 (float32r-limited).
"""
import os
import sys
import numpy as np
from contextlib import ExitStack

sys.path.insert(0, "/opt/trn_rl_repo")

import concourse.bass as bass  # noqa: E402,F401
import concourse.tile as tile  # noqa: E402
from concourse import mybir, bacc  # noqa: E402
from concourse.bass_utils import run_bass_kernel_spmd  # noqa: E402

P = 128
H = 768
KT = H // P          # 6 k-tiles over the hidden dim
D = 5                # num domains
NCORES = 8
HD = H // 2          # disc hidden = 384
BW = 256             # weight column-block width
EPS = 1e-5
F32 = mybir.dt.float32
F32R = mybir.dt.float32r


# ---------------------------------------------------------------- host: slot solver
def _solve_slots(counts):
    """Pick per-core slot capacities and assign (core, slot) -> domain.

    Every core runs the same program with S weight slots of sizes caps[s]
    (rows, multiples of 128).  Each of the 8*S slot instances is dedicated to
    one domain; a domain's rows are spread over its instances.  Returns
    (caps, assign) where assign[d] is a list of instance indices (c*S+s).
    """
    from itertools import product
    need = [int(c) for c in counts]

    def feasible(caps):
        S = len(caps)
        avail = [NCORES] * S
        doms = sorted(range(D), key=lambda d: -need[d])
        picks = {}

        def dfs(i):
            if i == len(doms):
                return True
            d = doms[i]
            c = need[d]
            if c == 0:
                picks[d] = [0] * S
                return dfs(i + 1)
            rng = [range(0, min(avail[s], 8) + 1) for s in range(S)]
            combos = []
            for n in product(*rng):
                cap = sum(n[s] * caps[s] for s in range(S))
                if cap >= c:
                    combos.append((cap, sum(n), n))
            combos.sort()
            for cap, tot, n in combos[:12]:
                if not all(n[s] <= avail[s] for s in range(S)):
                    continue
                for s in range(S):
                    avail[s] -= n[s]
                picks[d] = list(n)
                if dfs(i + 1):
                    return True
                for s in range(S):
                    avail[s] += n[s]
                del picks[d]
            return False

        return picks if dfs(0) else None

    for ntiles in range(8, 33):
        layouts = []
        for S in (2, 3):
            if S == 2:
                for a in range(ntiles - 2, 1, -1):
                    b = ntiles - a
                    if b < 2 or a < b:
                        continue
                    layouts.append((a * P, b * P))
            else:
                for a in range(ntiles - 4, 1, -1):
                    for b in range(min(a, ntiles - a - 2), 1, -1):
                        c = ntiles - a - b
                        if c < 2 or b < c:
                            continue
                        layouts.append((a * P, b * P, c * P))
        for caps in layouts:
            picks = feasible(caps)
            if picks is not None:
                S = len(caps)
                free = {s: [c * S + s for c in range(NCORES)] for s in range(S)}
                assign = {d: [] for d in range(D)}
                for d in range(D):
                    for s in range(S):
                        for _ in range(picks[d][s]):
                            assign[d].append(free[s].pop(0))
                return list(caps), assign
    raise RuntimeError("no feasible slot layout")


def _repack(W):
    """[Hin, Hout] -> [nmb*128, KTp*BW], partition-major 256-col blocks: one
    weight block is a single contiguous-per-partition DMA (6 KB lines)."""
    W = np.asarray(W, np.float32)
    Hin, Hout = W.shape
    KTp = Hin // P
    nmb = -(-Hout // BW)
    Wp = np.zeros((nmb, P, KTp, BW), np.float32)
    for mb in range(nmb):
        w = W[:, mb * BW:(mb + 1) * BW]           # [Hin, bw]
        Wp[mb, :, :, :w.shape[1]] = w.reshape(KTp, P, -1).transpose(1, 0, 2)
    return np.ascontiguousarray(Wp.reshape(nmb * P, KTp * BW))


# ---------------------------------------------------------------- device program
_prog_cache = {}


def _chunks(lo, hi, step=384):
    """Row chunks preferring 384 (best measured PE ns/row), every chunk >= 256
    and <= 512 (fp32r moving-operand limit), multiples of 128."""
    out = []
    x = lo
    while x < hi:
        rem = hi - x
        if rem == step or rem - step >= 256:
            sz = step
        elif rem <= 512:
            sz = rem
        else:
            sz = rem - 256
        out.append((x, sz))
        x += sz
    return out


def _build_program(caps):
    S = len(caps)
    R = sum(caps)
    starts = [sum(caps[:s]) for s in range(S)]

    nc = bacc.Bacc("TRN2", target_bir_lowering=False, debug=False)

    def din(name, shape, dt=F32R):
        return nc.dram_tensor(name, shape, dt, kind="ExternalInput").ap()

    def dout(name, shape, dt=F32):
        return nc.dram_tensor(name, shape, dt, kind="ExternalOutput").ap()

    def wshape(hin, hout):
        return [-(-hout // BW) * P, (hin // P) * BW]

    xT = din("xT", [H, R])
    eW1 = [din(f"eW1_{s}", wshape(H, H)) for s in range(S)]
    eW2 = [din(f"eW2_{s}", wshape(H, H)) for s in range(S)]
    eB1 = [din(f"eB1_{s}", [H], F32) for s in range(S)]
    eB2 = [din(f"eB2_{s}", [H], F32) for s in range(S)]
    eLNg = [din(f"eLNg_{s}", [H], F32) for s in range(S)]
    eLNb = [din(f"eLNb_{s}", [H], F32) for s in range(S)]
    iW1 = din("iW1", wshape(H, H))
    iW2 = din("iW2", wshape(H, H))
    iB1 = din("iB1", [H], F32)
    iB2 = din("iB2", [H], F32)
    iLNg = din("iLNg", [H], F32)
    iLNb = din("iLNb", [H], F32)
    discW1 = din("discW1", wshape(H, HD))
    discB1 = din("discB1", [HD], F32)
    discW2 = din("discW2", [HD, D])
    discB2 = din("discB2", [D], F32)
    clsW1a = din("clsW1a", wshape(H, H))   # rows 0:H of clsW1
    clsW1b = din("clsW1b", wshape(H, H))   # rows H:2H
    clsB1 = din("clsB1", [H], F32)
    clsW2 = din("clsW2", [H, 3])
    clsB2 = din("clsB2", [3], F32)
    selmaskT = din("selmaskT", [D, R])
    validT = din("validT", [1, R], F32)
    ones128 = din("ones128", [P, 1])
    ones5 = din("ones5", [D, 1])

    combT = dout("combT", [2 * H, R])      # rows 0:H = domain_feat^T, H:2H = inv^T
    aspT = dout("aspT", [3, R])
    dpT_out = dout("dpT", [D, R])
    lossv_out = dout("lossv", [1, R])

    with tile.TileContext(nc) as tc, ExitStack() as ctx:
        ctx.enter_context(nc.allow_low_precision(
            reason="float32r is fp32-width; rounding only feeds fp32r matmuls"))
        acts = ctx.enter_context(tc.tile_pool(name="acts", bufs=1))
        wpool = ctx.enter_context(tc.tile_pool(name="wpool", bufs=4))
        smalls = ctx.enter_context(tc.tile_pool(name="smalls", bufs=1))
        ephL = ctx.enter_context(tc.tile_pool(name="ephL", bufs=2))
        eph1 = ctx.enter_context(tc.tile_pool(name="eph1", bufs=2))
        stats = ctx.enter_context(tc.tile_pool(name="stats", bufs=2))
        lsepool = ctx.enter_context(tc.tile_pool(name="lsepool", bufs=1))
        ps_main = ctx.enter_context(tc.tile_pool(name="ps_main", bufs=6, space="PSUM"))
        ps_small = ctx.enter_context(tc.tile_pool(name="ps_small", bufs=2, space="PSUM"))

        # ---- persistent SBUF tensors
        x_sb = acts.tile([P, KT, R], F32R, tag="bigA")      # xT; slot later reused by hC
        hE = acts.tile([P, KT, R], F32R, tag="bigB")        # expert hidden; later hI
        dfT = acts.tile([P, KT, R], F32R, tag="dfT")        # expert out -> domain_feat^T
        invT = acts.tile([P, KT, R], F32R, tag="invT")
        hD_sb = acts.tile([P, KT // 2, R], F32R, tag="hD")  # disc hidden [384, R]

        def load_vec(ap, n=H):
            t = smalls.tile([P, n // P], F32, tag=f"v{ap.tensor.name}")
            nc.sync.dma_start(t[:], ap.rearrange("(o p) -> p o", p=P))
            return t

        eB1_sb = [load_vec(eB1[s]) for s in range(S)]
        eB2_sb = [load_vec(eB2[s]) for s in range(S)]
        eLNg_sb = [load_vec(eLNg[s]) for s in range(S)]
        eLNb_sb = [load_vec(eLNb[s]) for s in range(S)]
        iB1_sb = load_vec(iB1)
        iB2_sb = load_vec(iB2)
        iLNg_sb = load_vec(iLNg)
        iLNb_sb = load_vec(iLNb)
        clsB1_sb = load_vec(clsB1)
        discB1_sb = load_vec(discB1, HD)
        clsB2_sb = smalls.tile([3, 1], F32)
        nc.sync.dma_start(clsB2_sb[:], clsB2[:, None])
        discB2_sb = smalls.tile([D, 1], F32)
        nc.sync.dma_start(discB2_sb[:], discB2[:, None])
        ones128_sb = smalls.tile([P, 1], F32R)
        nc.sync.dma_start(ones128_sb[:], ones128)
        ones5_sb = smalls.tile([D, 1], F32R)
        nc.sync.dma_start(ones5_sb[:], ones5)
        eps_sb = smalls.tile([P, 1], F32)
        nc.vector.memset(eps_sb[:], EPS)
        wC2_sb = smalls.tile([P, KT, 3], F32R)
        for k in range(KT):
            nc.sync.dma_start(wC2_sb[:, k], clsW2[k * P:(k + 1) * P, :])
        wD2_sb = smalls.tile([P, KT // 2, D], F32R)
        for k in range(KT // 2):
            nc.sync.dma_start(wD2_sb[:, k], discW2[k * P:(k + 1) * P, :])

        # ---- load xT per (k, chunk): the first E1 matmul only waits for
        # one small piece instead of a full 4.6KB-per-partition row
        for (r0, rsz) in _chunks(0, R):
            for k in range(KT):
                nc.sync.dma_start(x_sb[:, k, r0:r0 + rsz],
                                  xT[k * P:(k + 1) * P, r0:r0 + rsz])

        def mm_layer(waps, srcs, dst, bias_sb, relu, rchunks, ncols=H, ktp=KT):
            """dst[:, :, rows] = (relu?)(sum_j waps[j].T @ srcs[j] + bias).

            waps: list of repacked weight DRAM APs; srcs: matching list of
            [128, ktp, R] SBUF sources (contractions concatenated).
            """
            nmb = -(-ncols // BW)
            for mb in range(nmb):
                bw = min(BW, ncols - mb * BW)
                wts = []
                for wap in waps:
                    wt = wpool.tile([P, ktp, BW], F32R, tag="w")
                    nc.sync.dma_start(
                        wt[:],
                        wap[mb * P:(mb + 1) * P, :].rearrange(
                            "p (k b) -> p k b", k=ktp))
                    wts.append(wt)
                for (r0, rsz) in rchunks:
                    for ml in range(bw // P):
                        m = mb * (BW // P) + ml
                        pt = ps_main.tile([P, 512], F32, tag="mm")
                        nmm = len(waps) * ktp
                        i = 0
                        for wt, src in zip(wts, srcs):
                            for k in range(ktp):
                                nc.tensor.matmul(
                                    pt[:, :rsz],
                                    wt[:, k, ml * P:(ml + 1) * P],
                                    src[:, k, r0:r0 + rsz],
                                    start=(i == 0), stop=(i == nmm - 1))
                                i += 1
                        if relu:
                            # ACT engine: relu(psum + bias) in one shot,
                            # keeps DVE free for LN work.
                            nc.scalar.activation(
                                dst[:, m, r0:r0 + rsz], pt[:, :rsz],
                                mybir.ActivationFunctionType.Relu,
                                bias=bias_sb[:, m:m + 1])
                        else:
                            nc.scalar.activation(
                                dst[:, m, r0:r0 + rsz], pt[:, :rsz],
                                mybir.ActivationFunctionType.Identity,
                                bias=bias_sb[:, m:m + 1])

        def layernorm(buf, g_sb, b_sb, rchunks, out_base=None, defer_beta=False):
            """In-place LN over the feature (partition x KT) axis of buf.

            The mean is already zero: the host pre-centers the columns of the
            preceding layer's weights/bias, so only the variance is needed.
            With defer_beta, gamma is pre-folded into the weights by the host
            and beta is applied only on the combT output copy — downstream
            matmul consumers read the beta-less tensor (their biases carry the
            host-folded W.T @ beta correction), halving the critical chain.
            """
            for (r0, rsz) in rchunks:
                s2 = ps_small.tile([1, 512], F32, tag="sm")
                for m in range(KT):
                    sq = ephL.tile([P, 512], F32R, tag="sq")
                    nc.vector.tensor_tensor(sq[:, :rsz], buf[:, m, r0:r0 + rsz],
                                            buf[:, m, r0:r0 + rsz],
                                            mybir.AluOpType.mult)
                    nc.tensor.matmul(s2[:, :rsz], ones128_sb[:], sq[:, :rsz],
                                     start=(m == 0), stop=(m == KT - 1))
                std = stats.tile([1, 512], F32, tag="stt")
                nc.scalar.activation(std[:, :rsz], s2[:, :rsz],
                                     mybir.ActivationFunctionType.Sqrt,
                                     bias=eps_sb[:1], scale=1.0 / H)
                rstd = stats.tile([1, 512], F32, tag="stt")
                nc.vector.reciprocal_approx_fast(rstd[:, :rsz], std[:, :rsz])
                rstdB = ephL.tile([P, 512], F32, tag="rstdB")
                nc.gpsimd.partition_broadcast(rstdB[:, :rsz], rstd[:, :rsz])
                for m in range(KT):
                    if defer_beta:
                        nc.vector.tensor_tensor(buf[:, m, r0:r0 + rsz],
                                                buf[:, m, r0:r0 + rsz],
                                                rstdB[:, :rsz],
                                                mybir.AluOpType.mult)
                        stg = ephL.tile([P, 512], F32, tag="stg")
                        nc.vector.tensor_scalar_add(stg[:, :rsz],
                                                    buf[:, m, r0:r0 + rsz],
                                                    b_sb[:, m:m + 1])
                        nc.sync.dma_start(
                            combT[out_base + m * P:out_base + (m + 1) * P,
                                  r0:r0 + rsz], stg[:, :rsz])
                        continue
                    nc.vector.scalar_tensor_tensor(buf[:, m, r0:r0 + rsz],
                                                   buf[:, m, r0:r0 + rsz],
                                                   g_sb[:, m:m + 1],
                                                   rstdB[:, :rsz],
                                                   mybir.AluOpType.mult,
                                                   mybir.AluOpType.mult)
                    nc.vector.tensor_scalar_add(buf[:, m, r0:r0 + rsz],
                                                buf[:, m, r0:r0 + rsz],
                                                b_sb[:, m:m + 1])
                    if out_base is not None:
                        # stream this chunk of combT out as soon as it's final
                        nc.sync.dma_start(
                            combT[out_base + m * P:out_base + (m + 1) * P,
                                  r0:r0 + rsz],
                            buf.bitcast(F32)[:, m, r0:r0 + rsz])

        all_chunks = _chunks(0, R)
        slot_chunks = [_chunks(starts[s], starts[s] + caps[s]) for s in range(S)]

        # ---- E1: expert hidden = relu(eW1_s.T @ x + eB1_s)
        for s in range(S):
            mm_layer([eW1[s]], [x_sb], hE, eB1_sb[s], True, slot_chunks[s])
        # ---- E2 -> dfT (pre-LN), LN per slot pipelined against next slot / I1
        for s in range(S):
            mm_layer([eW2[s]], [hE], dfT, eB2_sb[s], False, slot_chunks[s])
            layernorm(dfT, eLNg_sb[s], eLNb_sb[s], slot_chunks[s], out_base=0)

        # ---- I1: hI = relu(iW1.T @ x + iB1)   (hI reuses hE's slot)
        hI = acts.tile([P, KT, R], F32R, tag="bigB")
        mm_layer([iW1], [x_sb], hI, iB1_sb, True, all_chunks)

        # ---- I2 + LN -> invT
        mm_layer([iW2], [hI], invT, iB2_sb, False, all_chunks)
        layernorm(invT, iLNg_sb, iLNb_sb, all_chunks, out_base=H,
                  defer_beta=True)

        # ---- D1: disc hidden = relu(discW1.T @ invT + discB1)  [384, R]
        mm_layer([discW1], [invT], hD_sb, discB1_sb, True, all_chunks, ncols=HD)

        # ---- D2: dpT = discW2.T @ hD + discB2  [5, R]; per-row loss
        for (r0, rsz) in all_chunks:
            pt = ps_small.tile([D, 512], F32, tag="sm")
            for k in range(KT // 2):
                nc.tensor.matmul(pt[:, :rsz], wD2_sb[:, k],
                                 hD_sb[:, k, r0:r0 + rsz],
                                 start=(k == 0), stop=(k == KT // 2 - 1))
            dpch = ephL.tile([D, 512], F32, tag="dpch")
            nc.vector.tensor_scalar(dpch[:, :rsz], pt[:, :rsz],
                                    discB2_sb[:], None, mybir.AluOpType.add)
            nc.sync.dma_start(dpT_out[:, r0:r0 + rsz], dpch[:, :rsz])
            # loss: (lse - sel) * valid
            expdp = eph1.tile([D, 512], F32R, tag="expdp")
            nc.scalar.activation(expdp[:, :rsz], dpch[:, :rsz],
                                 mybir.ActivationFunctionType.Exp)
            se = ps_small.tile([1, 512], F32, tag="sm")
            nc.tensor.matmul(se[:, :rsz], ones5_sb[:], expdp[:, :rsz],
                             start=True, stop=True)
            lse = lsepool.tile([1, 512], F32, tag="lse")
            nc.scalar.activation(lse[:, :rsz], se[:, :rsz],
                                 mybir.ActivationFunctionType.Ln)
            selmch = eph1.tile([D, 512], F32R, tag="selmch")
            nc.sync.dma_start(selmch[:, :rsz], selmaskT[:, r0:r0 + rsz])
            selp = eph1.tile([D, 512], F32R, tag="selp")
            nc.vector.tensor_tensor(selp[:, :rsz], dpch.bitcast(F32R)[:, :rsz],
                                    selmch[:, :rsz], mybir.AluOpType.mult)
            ss = ps_small.tile([1, 512], F32, tag="sm")
            nc.tensor.matmul(ss[:, :rsz], ones5_sb[:], selp[:, :rsz],
                             start=True, stop=True)
            validch = eph1.tile([1, 512], F32, tag="validch")
            nc.sync.dma_start(validch[:, :rsz], validT[:, r0:r0 + rsz])
            lossch = eph1.tile([1, 512], F32, tag="lossch")
            nc.vector.tensor_tensor(lossch[:, :rsz], lse[:, :rsz], ss[:, :rsz],
                                    mybir.AluOpType.subtract)
            nc.vector.tensor_tensor(lossch[:, :rsz], lossch[:, :rsz],
                                    validch[:, :rsz], mybir.AluOpType.mult)
            nc.sync.dma_start(lossv_out[:, r0:r0 + rsz], lossch[:, :rsz])


        # ---- C1: cls hidden = relu(clsW1a.T @ dfT + clsW1b.T @ invT + clsB1)
        hC = acts.tile([P, KT, R], F32R, tag="bigA")
        mm_layer([clsW1a, clsW1b], [dfT, invT], hC, clsB1_sb, True, all_chunks)

        # ---- C2: aspT = clsW2.T @ hC + clsB2   [3, R]
        for (r0, rsz) in all_chunks:
            pt = ps_small.tile([3, 512], F32, tag="sm")
            for k in range(KT):
                nc.tensor.matmul(pt[:, :rsz], wC2_sb[:, k], hC[:, k, r0:r0 + rsz],
                                 start=(k == 0), stop=(k == KT - 1))
            aspch = ephL.tile([3, 512], F32, tag="aspch")
            nc.vector.tensor_scalar(aspch[:, :rsz], pt[:, :rsz],
                                    clsB2_sb[:], None, mybir.AluOpType.add)
            nc.sync.dma_start(aspT[:, r0:r0 + rsz], aspch[:, :rsz])

    nc.compile()
    return nc


# ---------------------------------------------------------------- host wrapper
def kernel(features, domain_ids, dW1, dB1, dW2, dB2, dLNg, dLNb,
           iW1, iB1, iW2, iB2, iLNg, iLNb,
           discW1, discB1, discW2, discB2,
           clsW1, clsB1, clsW2, clsB2):
    features = np.ascontiguousarray(np.asarray(features, dtype=np.float32))
    dom = np.asarray(domain_ids).astype(np.int64)
    B = features.shape[0]

    counts = np.bincount(dom, minlength=D)
    caps, assign = _solve_slots(counts)
    S = len(caps)
    R = sum(caps)
    starts = [sum(caps[:s]) for s in range(S)]

    key = tuple(caps)
    if key not in _prog_cache:
        _prog_cache[key] = _build_program(caps)
    nc = _prog_cache[key]

    # ---- scatter rows into (core, slot) instances
    orig = -np.ones((NCORES, R), dtype=np.int64)
    dom_of_slot = np.zeros((NCORES, S), dtype=np.int64)
    for d in range(D):
        rows = np.nonzero(dom == d)[0]
        off = 0
        for inst in assign[d]:
            c, s = divmod(inst, S)
            dom_of_slot[c, s] = d
            take = min(caps[s], len(rows) - off)
            if take > 0:
                orig[c, starts[s]:starts[s] + take] = rows[off:off + take]
            off += take
        assert off >= len(rows), f"domain {d} rows not fully placed"

    f32 = np.float32
    # Pre-center the columns of the pre-LayerNorm layers: subtracting the
    # per-row mean of h@W2+b2 is identical to using column-centered W2/b2,
    # which lets the device skip the LN mean entirely.
    iW2 = np.asarray(iW2, f32)
    iLNg = np.asarray(iLNg, f32)
    iLNb = np.asarray(iLNb, f32)
    iW2c = (iW2 - iW2.mean(axis=1, keepdims=True)) * iLNg[None, :]
    iB2c = (np.asarray(iB2, f32) - np.asarray(iB2, f32).mean()) * iLNg
    # invT's LN defers beta: downstream consumers get W.T @ beta folded into
    # their biases instead.
    discB1 = np.asarray(discB1, f32) + np.asarray(discW1, f32).T @ iLNb
    clsW1 = np.asarray(clsW1, f32)
    clsB1 = np.asarray(clsB1, f32) + clsW1[H:].T @ iLNb
    common = {
        "iW1": _repack(iW1), "iB1": np.ascontiguousarray(iB1, f32),
        "iW2": _repack(iW2c), "iB2": np.ascontiguousarray(iB2c, f32),
        "iLNg": np.ascontiguousarray(iLNg, f32),
        "iLNb": np.ascontiguousarray(iLNb, f32),
        "discW1": _repack(discW1),
        "discB1": np.ascontiguousarray(discB1, f32),
        "discW2": np.ascontiguousarray(discW2, f32),
        "discB2": np.ascontiguousarray(discB2, f32),
        "clsW1a": _repack(np.asarray(clsW1, f32)[:H]),
        "clsW1b": _repack(np.asarray(clsW1, f32)[H:]),
        "clsB1": np.ascontiguousarray(clsB1, f32),
        "clsW2": np.ascontiguousarray(clsW2, f32),
        "clsB2": np.ascontiguousarray(clsB2, f32),
        "ones128": np.ones((P, 1), f32),
        "ones5": np.ones((D, 1), f32),
    }
    dW1 = np.asarray(dW1, f32); dW2 = np.asarray(dW2, f32)
    dB1 = np.asarray(dB1, f32); dB2 = np.asarray(dB2, f32)
    dLNg = np.asarray(dLNg, f32); dLNb = np.asarray(dLNb, f32)
    dW2c = dW2 - dW2.mean(axis=2, keepdims=True)
    dB2c = dB2 - dB2.mean(axis=1, keepdims=True)
    eW1p = [_repack(dW1[d]) for d in range(D)]
    eW2p = [_repack(dW2c[d]) for d in range(D)]

    in_maps = []
    for c in range(NCORES):
        idx = orig[c]
        valid = idx >= 0
        xr = np.zeros((R, H), f32)
        xr[valid] = features[idx[valid]]
        selm = np.zeros((D, R), f32)
        vr = np.nonzero(valid)[0]
        selm[dom[idx[vr]], vr] = 1.0
        mm = dict(common)
        mm["xT"] = np.ascontiguousarray(xr.T)
        mm["selmaskT"] = selm
        mm["validT"] = valid.astype(f32).reshape(1, R)
        for s in range(S):
            d = int(dom_of_slot[c, s])
            mm[f"eW1_{s}"] = eW1p[d]
            mm[f"eW2_{s}"] = eW2p[d]
            mm[f"eB1_{s}"] = np.ascontiguousarray(dB1[d])
            mm[f"eB2_{s}"] = np.ascontiguousarray(dB2c[d])
            mm[f"eLNg_{s}"] = np.ascontiguousarray(dLNg[d])
            mm[f"eLNb_{s}"] = np.ascontiguousarray(dLNb[d])
        in_maps.append(mm)

    trace = os.environ.get("KERNEL_TRACE") == "1"
    if trace:
        _install_ntff_hook()
    res = run_bass_kernel_spmd(nc, in_maps, core_ids=list(range(NCORES)),
                               trace=trace)
    if trace and res.exec_time_ns is not None:
        print(f"HW exec time: {res.exec_time_ns} ns")

    # ---- unscatter
    aspect = np.zeros((B, 3), f32)
    combined = np.zeros((B, 2 * H), f32)
    dpred = np.zeros((B, D), f32)
    loss_sum = 0.0
    for c in range(NCORES):
        r = res.results[c]
        idx = orig[c]
        v = idx >= 0
        iv = idx[v]
        aspect[iv] = r["aspT"].T[v]
        combined[iv] = r["combT"].T[v]
        dpred[iv] = r["dpT"].T[v]
        loss_sum += float(r["lossv"].sum())
    domain_feat = combined[:, :H]
    inv = combined[:, H:]
    domain_loss = np.float32(loss_sum / B)
    return aspect, domain_feat, inv, combined, domain_loss, dpred


def _install_ntff_hook():
    import types
    if "antenv.axon_hooks" in sys.modules:
        return
    try:
        mod = types.ModuleType("antenv.axon_hooks")
        mod._hook = None
        mod.set_axon_ntff_profile_hook = lambda h: setattr(mod, "_hook", h)
        mod.get_axon_ntff_profile_hook = lambda: mod._hook
        sys.modules["antenv.axon_hooks"] = mod
        sys.path.insert(0, "/root/.axon_site")
        from trn_agent_boot.trn_boot import _ntff_profile_via_ctypes
        mod._hook = _ntff_profile_via_ctypes("/opt/axon/libaxon_pjrt.so")
    except Exception:
        sys.modules.pop("antenv.axon_hooks", None)


# revision 39
# speedup vs baseline: 1.0465x; 1.0465x over previous
"""Trainium2 Bass kernel for nn_CrossDomainAspectLabelPropagation.

Data-parallel MoE-routed implementation on 8 NeuronCores:
  - Host groups rows by domain and packs them into per-core "weight slots"
    (each (core, slot) processes rows of a single domain), so each core runs
    its expert MLP only once per row instead of densely over all 5 experts.
  - On-device layout is feature-major (activations stored transposed,
    [hidden, rows]): weights are the stationary matmul operand, activations
    the moving one, so no transposes are ever needed on device; the host
    pre/post-transposes.
  - All matmuls run in float32r (TF32-like, ~1e-4 rel err) at full PE rate.
  - LayerNorm needs no mean on device (the host pre-centers the columns of
    the preceding Linear, which is exact math); variance comes from
    squares (DVE) + ones-vector matmuls, rstd via ACT sqrt +
    reciprocal_approx_fast, broadcast with gpsimd partition_broadcast.
    For the invariant branch, gamma is host-folded into iW2 and beta is
    deferred to the combined-output copy (consumers carry W.T@beta in their
    biases), which halves the LN critical chain before disc/cls.
  - PSUM drains run on the ACT engine (fused bias+relu), squares/applies on
    DVE, broadcasts on GpSimd - balancing all four compute engines.
  - The domain loss is computed per-row on device; the host sums the 8
    partial vectors.

Measured on 8 axon-attached TRN2 NeuronCores: ~214 us HW exec time,
worst-output relative error ~3.9e-4 (float32r-limited).
"""
import os
import sys
import numpy as np
from contextlib import ExitStack

sys.path.insert(0, "/opt/trn_rl_repo")

import concourse.bass as bass  # noqa: E402,F401
import concourse.tile as tile  # noqa: E402
from concourse import mybir, bacc  # noqa: E402
from concourse.bass_utils import run_bass_kernel_spmd  # noqa: E402

P = 128
H = 768
KT = H // P          # 6 k-tiles over the hidden dim
D = 5                # num domains
NCORES = 8
HD = H // 2          # disc hidden = 384
BW = 256             # weight column-block width
EPS = 1e-5
F32 = mybir.dt.float32
F32R = mybir.dt.float32r


# ---------------------------------------------------------------- host: slot solver
def _solve_slots(counts):
    """Pick per-core slot capacities and assign (core, slot) -> domain.

    Every core runs the same program with S weight slots of sizes caps[s]
    (rows, multiples of 128).  Each of the 8*S slot instances is dedicated to
    one domain; a domain's rows are spread over its instances.  Returns
    (caps, assign) where assign[d] is a list of instance indices (c*S+s).
    """
    from itertools import product
    need = [int(c) for c in counts]

    def feasible(caps):
        S = len(caps)
        avail = [NCORES] * S
        doms = sorted(range(D), key=lambda d: -need[d])
        picks = {}

        def dfs(i):
            if i == len(doms):
                return True
            d = doms[i]
            c = need[d]
            if c == 0:
                picks[d] = [0] * S
                return dfs(i + 1)
            rng = [range(0, min(avail[s], 8) + 1) for s in range(S)]
            combos = []
            for n in product(*rng):
                cap = sum(n[s] * caps[s] for s in range(S))
                if cap >= c:
                    combos.append((cap, sum(n), n))
            combos.sort()
            for cap, tot, n in combos[:12]:
                if not all(n[s] <= avail[s] for s in range(S)):
                    continue
                for s in range(S):
                    avail[s] -= n[s]
                picks[d] = list(n)
                if dfs(i + 1):
                    return True
                for s in range(S):
                    avail[s] += n[s]
                del picks[d]
            return False

        return picks if dfs(0) else None

    for ntiles in range(8, 33):
        layouts = []
        for S in (2, 3):
            if S == 2:
                for a in range(ntiles - 2, 1, -1):
                    b = ntiles - a
                    if b < 2 or a < b:
                        continue
                    layouts.append((a * P, b * P))
            else:
                for a in range(ntiles - 4, 1, -1):
                    for b in range(min(a, ntiles - a - 2), 1, -1):
                        c = ntiles - a - b
                        if c < 2 or b < c:
                            continue
                        layouts.append((a * P, b * P, c * P))
        for caps in layouts:
            picks = feasible(caps)
            if picks is not None:
                S = len(caps)
                free = {s: [c * S + s for c in range(NCORES)] for s in range(S)}
                assign = {d: [] for d in range(D)}
                for d in range(D):
                    for s in range(S):
                        for _ in range(picks[d][s]):
                            assign[d].append(free[s].pop(0))
                return list(caps), assign
    raise RuntimeError("no feasible slot layout")


def _repack(W):
    """[Hin, Hout] -> [nmb*128, KTp*BW], partition-major 256-col blocks: one
    weight block is a single contiguous-per-partition DMA (6 KB lines)."""
    W = np.asarray(W, np.float32)
    Hin, Hout = W.shape
    KTp = Hin // P
    nmb = -(-Hout // BW)
    Wp = np.zeros((nmb, P, KTp, BW), np.float32)
    for mb in range(nmb):
        w = W[:, mb * BW:(mb + 1) * BW]           # [Hin, bw]
        Wp[mb, :, :, :w.shape[1]] = w.reshape(KTp, P, -1).transpose(1, 0, 2)
    return np.ascontiguousarray(Wp.reshape(nmb * P, KTp * BW))


# ---------------------------------------------------------------- device program
_prog_cache = {}


def _chunks(lo, hi, step=384):
    """Row chunks preferring 384 (best measured PE ns/row), every chunk >= 256
    and <= 512 (fp32r moving-operand limit), multiples of 128."""
    out = []
    x = lo
    while x < hi:
        rem = hi - x
        if rem == step or rem - step >= 256:
            sz = step
        elif rem <= 512:
            sz = rem
        else:
            sz = rem - 256
        out.append((x, sz))
        x += sz
    return out


def _build_program(caps):
    S = len(caps)
    R = sum(caps)
    starts = [sum(caps[:s]) for s in range(S)]

    nc = bacc.Bacc("TRN2", target_bir_lowering=False, debug=False)

    def din(name, shape, dt=F32R):
        return nc.dram_tensor(name, shape, dt, kind="ExternalInput").ap()

    def dout(name, shape, dt=F32):
        return nc.dram_tensor(name, shape, dt, kind="ExternalOutput").ap()

    def wshape(hin, hout):
        return [-(-hout // BW) * P, (hin // P) * BW]

    xT = din("xT", [H, R])
    eW1 = [din(f"eW1_{s}", wshape(H, H)) for s in range(S)]
    eW2 = [din(f"eW2_{s}", wshape(H, H)) for s in range(S)]
    eB1 = [din(f"eB1_{s}", [H], F32) for s in range(S)]
    eB2 = [din(f"eB2_{s}", [H], F32) for s in range(S)]
    eLNg = [din(f"eLNg_{s}", [H], F32) for s in range(S)]
    eLNb = [din(f"eLNb_{s}", [H], F32) for s in range(S)]
    iW1 = din("iW1", wshape(H, H))
    iW2 = din("iW2", wshape(H, H))
    iB1 = din("iB1", [H], F32)
    iB2 = din("iB2", [H], F32)
    iLNg = din("iLNg", [H], F32)
    iLNb = din("iLNb", [H], F32)
    discW1 = din("discW1", wshape(H, HD))
    discB1 = din("discB1", [HD], F32)
    discW2 = din("discW2", [HD, D])
    discB2 = din("discB2", [D], F32)
    clsW1a = din("clsW1a", wshape(H, H))   # rows 0:H of clsW1
    clsW1b = din("clsW1b", wshape(H, H))   # rows H:2H
    clsB1 = din("clsB1", [H], F32)
    clsW2 = din("clsW2", [H, 3])
    clsB2 = din("clsB2", [3], F32)
    selmaskT = din("selmaskT", [D, R])
    validT = din("validT", [1, R], F32)
    ones128 = din("ones128", [P, 1])
    ones5 = din("ones5", [D, 1])

    combT = dout("combT", [2 * H, R])      # rows 0:H = domain_feat^T, H:2H = inv^T
    aspT = dout("aspT", [3, R])
    dpT_out = dout("dpT", [D, R])
    lossv_out = dout("lossv", [1, R])

    with tile.TileContext(nc) as tc, ExitStack() as ctx:
        ctx.enter_context(nc.allow_low_precision(
            reason="float32r is fp32-width; rounding only feeds fp32r matmuls"))
        acts = ctx.enter_context(tc.tile_pool(name="acts", bufs=1))
        wpool = ctx.enter_context(tc.tile_pool(name="wpool", bufs=4))
        smalls = ctx.enter_context(tc.tile_pool(name="smalls", bufs=1))
        ephL = ctx.enter_context(tc.tile_pool(name="ephL", bufs=2))
        eph1 = ctx.enter_context(tc.tile_pool(name="eph1", bufs=2))
        stats = ctx.enter_context(tc.tile_pool(name="stats", bufs=2))
        lsepool = ctx.enter_context(tc.tile_pool(name="lsepool", bufs=1))
        ps_main = ctx.enter_context(tc.tile_pool(name="ps_main", bufs=6, space="PSUM"))
        ps_small = ctx.enter_context(tc.tile_pool(name="ps_small", bufs=2, space="PSUM"))

        # ---- persistent SBUF tensors
        x_sb = acts.tile([P, KT, R], F32R, tag="bigA")      # xT; slot later reused by hC
        hE = acts.tile([P, KT, R], F32R, tag="bigB")        # expert hidden; later hI
        dfT = acts.tile([P, KT, R], F32R, tag="dfT")        # expert out -> domain_feat^T
        invT = acts.tile([P, KT, R], F32R, tag="invT")
        hD_sb = acts.tile([P, KT // 2, R], F32R, tag="hD")  # disc hidden [384, R]

        # xT first, on the GpSimd DMA queues so E1's inputs are at the
        # front of their rings instead of behind dozens of small loads
        for (r0, rsz) in _chunks(0, R):
            for k in range(KT):
                nc.gpsimd.dma_start(x_sb[:, k, r0:r0 + rsz],
                                    xT[k * P:(k + 1) * P, r0:r0 + rsz])

        def load_vec(ap, n=H):
            t = smalls.tile([P, n // P], F32, tag=f"v{ap.tensor.name}")
            nc.sync.dma_start(t[:], ap.rearrange("(o p) -> p o", p=P))
            return t

        eB1_sb = [load_vec(eB1[s]) for s in range(S)]
        eB2_sb = [load_vec(eB2[s]) for s in range(S)]
        eLNg_sb = [load_vec(eLNg[s]) for s in range(S)]
        eLNb_sb = [load_vec(eLNb[s]) for s in range(S)]
        iB1_sb = load_vec(iB1)
        iB2_sb = load_vec(iB2)
        iLNg_sb = load_vec(iLNg)
        iLNb_sb = load_vec(iLNb)
        clsB1_sb = load_vec(clsB1)
        discB1_sb = load_vec(discB1, HD)
        clsB2_sb = smalls.tile([3, 1], F32)
        nc.sync.dma_start(clsB2_sb[:], clsB2[:, None])
        discB2_sb = smalls.tile([D, 1], F32)
        nc.sync.dma_start(discB2_sb[:], discB2[:, None])
        ones128_sb = smalls.tile([P, 1], F32R)
        nc.sync.dma_start(ones128_sb[:], ones128)
        ones5_sb = smalls.tile([D, 1], F32R)
        nc.sync.dma_start(ones5_sb[:], ones5)
        eps_sb = smalls.tile([P, 1], F32)
        nc.vector.memset(eps_sb[:], EPS)
        wC2_sb = smalls.tile([P, KT, 3], F32R)
        for k in range(KT):
            nc.sync.dma_start(wC2_sb[:, k], clsW2[k * P:(k + 1) * P, :])
        wD2_sb = smalls.tile([P, KT // 2, D], F32R)
        for k in range(KT // 2):
            nc.sync.dma_start(wD2_sb[:, k], discW2[k * P:(k + 1) * P, :])


        def mm_layer(waps, srcs, dst, bias_sb, relu, rchunks, ncols=H, ktp=KT):
            """dst[:, :, rows] = (relu?)(sum_j waps[j].T @ srcs[j] + bias).

            waps: list of repacked weight DRAM APs; srcs: matching list of
            [128, ktp, R] SBUF sources (contractions concatenated).
            """
            nmb = -(-ncols // BW)
            for mb in range(nmb):
                bw = min(BW, ncols - mb * BW)
                wts = []
                for wap in waps:
                    wt = wpool.tile([P, ktp, BW], F32R, tag="w")
                    nc.sync.dma_start(
                        wt[:],
                        wap[mb * P:(mb + 1) * P, :].rearrange(
                            "p (k b) -> p k b", k=ktp))
                    wts.append(wt)
                for (r0, rsz) in rchunks:
                    for ml in range(bw // P):
                        m = mb * (BW // P) + ml
                        pt = ps_main.tile([P, 512], F32, tag="mm")
                        nmm = len(waps) * ktp
                        i = 0
                        for wt, src in zip(wts, srcs):
                            for k in range(ktp):
                                nc.tensor.matmul(
                                    pt[:, :rsz],
                                    wt[:, k, ml * P:(ml + 1) * P],
                                    src[:, k, r0:r0 + rsz],
                                    start=(i == 0), stop=(i == nmm - 1))
                                i += 1
                        if relu:
                            # ACT engine: relu(psum + bias) in one shot,
                            # keeps DVE free for LN work.
                            nc.scalar.activation(
                                dst[:, m, r0:r0 + rsz], pt[:, :rsz],
                                mybir.ActivationFunctionType.Relu,
                                bias=bias_sb[:, m:m + 1])
                        else:
                            nc.scalar.activation(
                                dst[:, m, r0:r0 + rsz], pt[:, :rsz],
                                mybir.ActivationFunctionType.Identity,
                                bias=bias_sb[:, m:m + 1])

        def layernorm(buf, g_sb, b_sb, rchunks, out_base=None, defer_beta=False):
            """In-place LN over the feature (partition x KT) axis of buf.

            The mean is already zero: the host pre-centers the columns of the
            preceding layer's weights/bias, so only the variance is needed.
            With defer_beta, gamma is pre-folded into the weights by the host
            and beta is applied only on the combT output copy — downstream
            matmul consumers read the beta-less tensor (their biases carry the
            host-folded W.T @ beta correction), halving the critical chain.
            """
            for (r0, rsz) in rchunks:
                s2 = ps_small.tile([1, 512], F32, tag="sm")
                for m in range(KT):
                    sq = ephL.tile([P, 512], F32R, tag="sq")
                    nc.vector.tensor_tensor(sq[:, :rsz], buf[:, m, r0:r0 + rsz],
                                            buf[:, m, r0:r0 + rsz],
                                            mybir.AluOpType.mult)
                    nc.tensor.matmul(s2[:, :rsz], ones128_sb[:], sq[:, :rsz],
                                     start=(m == 0), stop=(m == KT - 1))
                std = stats.tile([1, 512], F32, tag="stt")
                nc.scalar.activation(std[:, :rsz], s2[:, :rsz],
                                     mybir.ActivationFunctionType.Sqrt,
                                     bias=eps_sb[:1], scale=1.0 / H)
                rstd = stats.tile([1, 512], F32, tag="stt")
                nc.vector.reciprocal_approx_fast(rstd[:, :rsz], std[:, :rsz])
                rstdB = ephL.tile([P, 512], F32, tag="rstdB")
                nc.gpsimd.partition_broadcast(rstdB[:, :rsz], rstd[:, :rsz])
                for m in range(KT):
                    if defer_beta:
                        nc.vector.tensor_tensor(buf[:, m, r0:r0 + rsz],
                                                buf[:, m, r0:r0 + rsz],
                                                rstdB[:, :rsz],
                                                mybir.AluOpType.mult)
                        stg = ephL.tile([P, 512], F32, tag="stg")
                        nc.vector.tensor_scalar_add(stg[:, :rsz],
                                                    buf[:, m, r0:r0 + rsz],
                                                    b_sb[:, m:m + 1])
                        nc.sync.dma_start(
                            combT[out_base + m * P:out_base + (m + 1) * P,
                                  r0:r0 + rsz], stg[:, :rsz])
                        continue
                    nc.vector.scalar_tensor_tensor(buf[:, m, r0:r0 + rsz],
                                                   buf[:, m, r0:r0 + rsz],
                                                   g_sb[:, m:m + 1],
                                                   rstdB[:, :rsz],
                                                   mybir.AluOpType.mult,
                                                   mybir.AluOpType.mult)
                    nc.vector.tensor_scalar_add(buf[:, m, r0:r0 + rsz],
                                                buf[:, m, r0:r0 + rsz],
                                                b_sb[:, m:m + 1])
                    if out_base is not None:
                        # stream this chunk of combT out as soon as it's final
                        nc.sync.dma_start(
                            combT[out_base + m * P:out_base + (m + 1) * P,
                                  r0:r0 + rsz],
                            buf.bitcast(F32)[:, m, r0:r0 + rsz])

        all_chunks = _chunks(0, R)
        slot_chunks = [_chunks(starts[s], starts[s] + caps[s]) for s in range(S)]

        # ---- E1: expert hidden = relu(eW1_s.T @ x + eB1_s)
        for s in range(S):
            mm_layer([eW1[s]], [x_sb], hE, eB1_sb[s], True, slot_chunks[s])
        # ---- E2 -> dfT (pre-LN), LN per slot pipelined against next slot / I1
        for s in range(S):
            mm_layer([eW2[s]], [hE], dfT, eB2_sb[s], False, slot_chunks[s])
            layernorm(dfT, eLNg_sb[s], eLNb_sb[s], slot_chunks[s], out_base=0)

        # ---- I1: hI = relu(iW1.T @ x + iB1)   (hI reuses hE's slot)
        hI = acts.tile([P, KT, R], F32R, tag="bigB")
        mm_layer([iW1], [x_sb], hI, iB1_sb, True, all_chunks)

        # ---- I2 + LN -> invT
        mm_layer([iW2], [hI], invT, iB2_sb, False, all_chunks)
        layernorm(invT, iLNg_sb, iLNb_sb, all_chunks, out_base=H,
                  defer_beta=True)

        # ---- D1: disc hidden = relu(discW1.T @ invT + discB1)  [384, R]
        mm_layer([discW1], [invT], hD_sb, discB1_sb, True, all_chunks, ncols=HD)

        # ---- D2: dpT = discW2.T @ hD + discB2  [5, R]; per-row loss
        for (r0, rsz) in all_chunks:
            pt = ps_small.tile([D, 512], F32, tag="sm")
            for k in range(KT // 2):
                nc.tensor.matmul(pt[:, :rsz], wD2_sb[:, k],
                                 hD_sb[:, k, r0:r0 + rsz],
                                 start=(k == 0), stop=(k == KT // 2 - 1))
            dpch = ephL.tile([D, 512], F32, tag="dpch")
            nc.vector.tensor_scalar(dpch[:, :rsz], pt[:, :rsz],
                                    discB2_sb[:], None, mybir.AluOpType.add)
            nc.sync.dma_start(dpT_out[:, r0:r0 + rsz], dpch[:, :rsz])
            # loss: (lse - sel) * valid
            expdp = eph1.tile([D, 512], F32R, tag="expdp")
            nc.scalar.activation(expdp[:, :rsz], dpch[:, :rsz],
                                 mybir.ActivationFunctionType.Exp)
            se = ps_small.tile([1, 512], F32, tag="sm")
            nc.tensor.matmul(se[:, :rsz], ones5_sb[:], expdp[:, :rsz],
                             start=True, stop=True)
            lse = lsepool.tile([1, 512], F32, tag="lse")
            nc.scalar.activation(lse[:, :rsz], se[:, :rsz],
                                 mybir.ActivationFunctionType.Ln)
            selmch = eph1.tile([D, 512], F32R, tag="selmch")
            nc.sync.dma_start(selmch[:, :rsz], selmaskT[:, r0:r0 + rsz])
            selp = eph1.tile([D, 512], F32R, tag="selp")
            nc.vector.tensor_tensor(selp[:, :rsz], dpch.bitcast(F32R)[:, :rsz],
                                    selmch[:, :rsz], mybir.AluOpType.mult)
            ss = ps_small.tile([1, 512], F32, tag="sm")
            nc.tensor.matmul(ss[:, :rsz], ones5_sb[:], selp[:, :rsz],
                             start=True, stop=True)
            validch = eph1.tile([1, 512], F32, tag="validch")
            nc.sync.dma_start(validch[:, :rsz], validT[:, r0:r0 + rsz])
            lossch = eph1.tile([1, 512], F32, tag="lossch")
            nc.vector.tensor_tensor(lossch[:, :rsz], lse[:, :rsz], ss[:, :rsz],
                                    mybir.AluOpType.subtract)
            nc.vector.tensor_tensor(lossch[:, :rsz], lossch[:, :rsz],
                                    validch[:, :rsz], mybir.AluOpType.mult)
            nc.sync.dma_start(lossv_out[:, r0:r0 + rsz], lossch[:, :rsz])


        # ---- C1: cls hidden = relu(clsW1a.T @ dfT + clsW1b.T @ invT + clsB1)
        hC = acts.tile([P, KT, R], F32R, tag="bigA")
        mm_layer([clsW1a, clsW1b], [dfT, invT], hC, clsB1_sb, True, all_chunks)

        # ---- C2: aspT = clsW2.T @ hC + clsB2   [3, R]
        for (r0, rsz) in all_chunks:
            pt = ps_small.tile([3, 512], F32, tag="sm")
            for k in range(KT):
                nc.tensor.matmul(pt[:, :rsz], wC2_sb[:, k], hC[:, k, r0:r0 + rsz],
                                 start=(k == 0), stop=(k == KT - 1))
            aspch = ephL.tile([3, 512], F32, tag="aspch")
            nc.vector.tensor_scalar(aspch[:, :rsz], pt[:, :rsz],
                                    clsB2_sb[:], None, mybir.AluOpType.add)
            nc.sync.dma_start(aspT[:, r0:r0 + rsz], aspch[:, :rsz])

    nc.compile()
    return nc


# ---------------------------------------------------------------- host wrapper
def kernel(features, domain_ids, dW1, dB1, dW2, dB2, dLNg, dLNb,
           iW1, iB1, iW2, iB2, iLNg, iLNb,
           discW1, discB1, discW2, discB2,
           clsW1, clsB1, clsW2, clsB2):
    features = np.ascontiguousarray(np.asarray(features, dtype=np.float32))
    dom = np.asarray(domain_ids).astype(np.int64)
    B = features.shape[0]

    counts = np.bincount(dom, minlength=D)
    caps, assign = _solve_slots(counts)
    S = len(caps)
    R = sum(caps)
    starts = [sum(caps[:s]) for s in range(S)]

    key = tuple(caps)
    if key not in _prog_cache:
        _prog_cache[key] = _build_program(caps)
    nc = _prog_cache[key]

    # ---- scatter rows into (core, slot) instances
    orig = -np.ones((NCORES, R), dtype=np.int64)
    dom_of_slot = np.zeros((NCORES, S), dtype=np.int64)
    for d in range(D):
        rows = np.nonzero(dom == d)[0]
        off = 0
        for inst in assign[d]:
            c, s = divmod(inst, S)
            dom_of_slot[c, s] = d
            take = min(caps[s], len(rows) - off)
            if take > 0:
                orig[c, starts[s]:starts[s] + take] = rows[off:off + take]
            off += take
        assert off >= len(rows), f"domain {d} rows not fully placed"

    f32 = np.float32
    # Pre-center the columns of the pre-LayerNorm layers: subtracting the
    # per-row mean of h@W2+b2 is identical to using column-centered W2/b2,
    # which lets the device skip the LN mean entirely.
    iW2 = np.asarray(iW2, f32)
    iLNg = np.asarray(iLNg, f32)
    iLNb = np.asarray(iLNb, f32)
    iW2c = (iW2 - iW2.mean(axis=1, keepdims=True)) * iLNg[None, :]
    iB2c = (np.asarray(iB2, f32) - np.asarray(iB2, f32).mean()) * iLNg
    # invT's LN defers beta: downstream consumers get W.T @ beta folded into
    # their biases instead.
    discB1 = np.asarray(discB1, f32) + np.asarray(discW1, f32).T @ iLNb
    clsW1 = np.asarray(clsW1, f32)
    clsB1 = np.asarray(clsB1, f32) + clsW1[H:].T @ iLNb
    common = {
        "iW1": _repack(iW1), "iB1": np.ascontiguousarray(iB1, f32),
        "iW2": _repack(iW2c), "iB2": np.ascontiguousarray(iB2c, f32),
        "iLNg": np.ascontiguousarray(iLNg, f32),
        "iLNb": np.ascontiguousarray(iLNb, f32),
        "discW1": _repack(discW1),
        "discB1": np.ascontiguousarray(discB1, f32),
        "discW2": np.ascontiguousarray(discW2, f32),
        "discB2": np.ascontiguousarray(discB2, f32),
        "clsW1a": _repack(np.asarray(clsW1, f32)[:H]),
        "clsW1b": _repack(np.asarray(clsW1, f32)[H:]),
        "clsB1": np.ascontiguousarray(clsB1, f32),
        "clsW2": np.ascontiguousarray(clsW2, f32),
        "clsB2": np.ascontiguousarray(clsB2, f32),
        "ones128": np.ones((P, 1), f32),
        "ones5": np.ones((D, 1), f32),
    }
    dW1 = np.asarray(dW1, f32); dW2 = np.asarray(dW2, f32)
    dB1 = np.asarray(dB1, f32); dB2 = np.asarray(dB2, f32)
    dLNg = np.asarray(dLNg, f32); dLNb = np.asarray(dLNb, f32)
    dW2c = dW2 - dW2.mean(axis=2, keepdims=True)
    dB2c = dB2 - dB2.mean(axis=1, keepdims=True)
    eW1p = [_repack(dW1[d]) for d in range(D)]
    eW2p = [_repack(dW2c[d]) for d in range(D)]

    in_maps = []
    for c in range(NCORES):
        idx = orig[c]
        valid = idx >= 0
        xr = np.zeros((R, H), f32)
        xr[valid] = features[idx[valid]]
        selm = np.zeros((D, R), f32)
        vr = np.nonzero(valid)[0]
        selm[dom[idx[vr]], vr] = 1.0
        mm = dict(common)
        mm["xT"] = np.ascontiguousarray(xr.T)
        mm["selmaskT"] = selm
        mm["validT"] = valid.astype(f32).reshape(1, R)
        for s in range(S):
            d = int(dom_of_slot[c, s])
            mm[f"eW1_{s}"] = eW1p[d]
            mm[f"eW2_{s}"] = eW2p[d]
            mm[f"eB1_{s}"] = np.ascontiguousarray(dB1[d])
            mm[f"eB2_{s}"] = np.ascontiguousarray(dB2c[d])
            mm[f"eLNg_{s}"] = np.ascontiguousarray(dLNg[d])
            mm[f"eLNb_{s}"] = np.ascontiguousarray(dLNb[d])
        in_maps.append(mm)

    trace = os.environ.get("KERNEL_TRACE") == "1"
    if trace:
        _install_ntff_hook()
    res = run_bass_kernel_spmd(nc, in_maps, core_ids=list(range(NCORES)),
                               trace=trace)
    if trace and res.exec_time_ns is not None:
        print(f"HW exec time: {res.exec_time_ns} ns")

    # ---- unscatter
    aspect = np.zeros((B, 3), f32)
    combined = np.zeros((B, 2 * H), f32)
    dpred = np.zeros((B, D), f32)
    loss_sum = 0.0
    for c in range(NCORES):
        r = res.results[c]
        idx = orig[c]
        v = idx >= 0
        iv = idx[v]
        aspect[iv] = r["aspT"].T[v]
        combined[iv] = r["combT"].T[v]
        dpred[iv] = r["dpT"].T[v]
        loss_sum += float(r["lossv"].sum())
    domain_feat = combined[:, :H]
    inv = combined[:, H:]
    domain_loss = np.float32(loss_sum / B)
    return aspect, domain_feat, inv, combined, domain_loss, dpred


def _install_ntff_hook():
    import types
    if "antenv.axon_hooks" in sys.modules:
        return
    try:
        mod = types.ModuleType("antenv.axon_hooks")
        mod._hook = None
        mod.set_axon_ntff_profile_hook = lambda h: setattr(mod, "_hook", h)
        mod.get_axon_ntff_profile_hook = lambda: mod._hook
        sys.modules["antenv.axon_hooks"] = mod
        sys.path.insert(0, "/root/.axon_site")
        from trn_agent_boot.trn_boot import _ntff_profile_via_ctypes
        mod._hook = _ntff_profile_via_ctypes("/opt/axon/libaxon_pjrt.so")
    except Exception:
        sys.modules.pop("antenv.axon_hooks", None)
